# revision 18
# baseline (speedup 1.0000x reference)
"""DINOv2 self-attention (QKV projection + SDPA, no out-proj) on 8 Trainium2
NeuronCores.

Sharding: pure data-parallel over batch (B=8 -> one batch element per core);
no cross-core communication.

Host-side prep inside kernel(): transpose hidden_states to x.T per batch and
pack W as W.T = [Wq.T | Wk.T | Wv.T], so every on-chip matmul operand already
has its contraction dim on the partition axis.

Per-core program (S=1370, D=1024, H=16, hd=64), all matmuls in float32r
(TF32-like, full PE rate at moving-dim >= 256):
  1. qT = (x Wq^T + bq)^T, kT likewise: out.T = W.T^T @ x.T with W.T tiles
     stationary; bias added on DVE during PSUM->SBUF copy.  Layout [o, s]
     puts head_dim on partitions for attention.
  2. v in standard layout [s, o] (x.T tiles stationary, W.T moving), written
     interleaved with a ones-column per head: v_ext[:, t, h*65+64] = 1 so the
     softmax denominator falls out of the ctx matmul as an extra output row.
  3. Per head-pair (two heads share an o-tile at partition 0/64 -> concurrent
     PE row-groups), per sq-chunk (<=512): scoresT[sk, sq] = kT^T @ qT,
     exp via ACT with fused 1/8 scale (softmax max-subtraction skipped:
     |scores/8| <= ~6), ctx.T[65, sq] accumulated over the 11 sk tiles with
     [v | 1] stationary.
  4. ctx.T (+denominator row) transposed back with PE transpose per 128-wide
     sub-tile, then out[:, h*64:h*64+64] = ctx * (1/denom) on DVE.
"""

import numpy as np
from contextlib import ExitStack

import concourse.bass as bass
import concourse.bacc as bacc
import concourse.tile as tile
from concourse import mybir
from concourse import bass_utils
from concourse.masks import make_identity

S, D, H, HD = 1370, 1024, 16, 64
F32 = mybir.dt.float32
F32R = mybir.dt.float32r
ND = D // 128                      # 8 contraction tiles
NO = D // 128                      # 8 output tiles per projection
NT = (S + 127) // 128              # 11 token tiles
TSZ = [min(128, S - i * 128) for i in range(NT)]
CHUNKS = [(0, 512), (512, 512), (1024, S - 1024)]
EXP = mybir.ActivationFunctionType.Exp


def _body(tc, xT, wT, bT, bvb, out, reps=1):
    nc = tc.nc
    with ExitStack() as ctx:
        const = ctx.enter_context(tc.tile_pool(name="const", bufs=1))
        ident = const.tile([128, 128], F32)
        make_identity(nc, ident)
        bT_sb = const.tile([128, 24], F32)
        nc.sync.dma_start(bT_sb[:], bT[:])

        qk_pool = ctx.enter_context(tc.tile_pool(name="qkT", bufs=1))
        vext_pool = ctx.enter_context(tc.tile_pool(name="vext", bufs=1))
        for _rep in range(reps):
            _one_pass(tc, ctx, qk_pool, vext_pool, ident, bT_sb, bvb,
                      xT, wT, out)


def _one_pass(tc, ctx, qk_pool, vext_pool, ident, bT_sb, bvb, xT, wT, out):
        nc = tc.nc
        qT = qk_pool.tile([128, NO, S], F32R, tag="qT", name="qT")
        kT = qk_pool.tile([128, NO, S], F32R, tag="kT", name="kT")
        v_ext = vext_pool.tile([128, NT, H * 65], F32R, tag="vext", name="v_ext")
        # ones columns (h*65+64) for the fused softmax denominator; memset
        # can't produce f32r, so synthesize 1.0 as in0*0 + 1 on DVE
        for t in range(NT):
            ones_view = v_ext[:, t, :].rearrange("p (h e) -> p h e", e=65)[:, :, 64]
            nc.vector.tensor_scalar(
                ones_view, bT_sb[:, 0:16],
                0.0, 1.0, mybir.AluOpType.mult, mybir.AluOpType.add)

        with ExitStack() as s1:
            xt_pool = s1.enter_context(tc.tile_pool(name="xt", bufs=1))
            xt = xt_pool.tile([128, ND, S], F32R)
            for d in range(ND):
                nc.sync.dma_start(xt[:, d, :], xT[d * 128:(d + 1) * 128, :])

            # ---- v = x @ Wv^T + bv, scattered into v_ext ----
            with ExitStack() as s2:
                wv_pool = s2.enter_context(tc.tile_pool(name="wv", bufs=1))
                bvb_sb = wv_pool.tile([128, D], F32, tag="bvb", name="bvb_sb")
                nc.sync.dma_start(bvb_sb[:], bvb[:])
                psv = s2.enter_context(
                    tc.tile_pool(name="psv", bufs=4, space="PSUM"))
                for half in range(2):
                    wv = wv_pool.tile([128, ND, 512], F32R, tag="wv", name="wv")
                    for d in range(ND):
                        c = 2 * D + half * 512
                        nc.sync.dma_start(
                            wv[:, d, :], wT[d * 128:(d + 1) * 128, c:c + 512])
                    for t in range(NT):
                        tsz = TSZ[t]
                        ps = psv.tile([128, 512], F32, tag="psv", name="psv")
                        for d in range(ND):
                            nc.tensor.matmul(
                                ps[:tsz, :], xt[:, d, t * 128:t * 128 + tsz],
                                wv[:, d, :], start=(d == 0), stop=(d == ND - 1))
                        dst = v_ext[:tsz, t, :].rearrange(
                            "p (h e) -> p h e", e=65)[:, half * 8:(half + 1) * 8, 0:64]
                        src = ps[:tsz, :].rearrange("p (h e) -> p h e", e=64)
                        bias = bvb_sb[:tsz, half * 512:(half + 1) * 512].rearrange(
                            "p (h e) -> p h e", e=64)
                        nc.vector.tensor_add(dst, src, bias)

            # ---- qT / kT projections (W loaded 2 o-tiles per DMA) ----
            with ExitStack() as s3:
                wqk_pool = s3.enter_context(tc.tile_pool(name="wqk", bufs=9))
                psqk = s3.enter_context(
                    tc.tile_pool(name="psqk", bufs=4, space="PSUM"))
                for og in range(0, NO, 2):
                    for proj in (1, 0):
                        dstT = qT if proj == 0 else kT
                        ws = []
                        for d in range(ND):
                            w = wqk_pool.tile([128, 256], F32R,
                                              tag="wqk", name="wqk")
                            c = proj * D + og * 128
                            nc.sync.dma_start(
                                w[:], wT[d * 128:(d + 1) * 128, c:c + 256])
                            ws.append(w)
                        for oo in range(2):
                            o = og + oo
                            for (c0, cw) in CHUNKS:
                                ps = psqk.tile([128, 512], F32,
                                               tag="psqk", name="psqk")
                                for d in range(ND):
                                    nc.tensor.matmul(
                                        ps[:, :cw],
                                        ws[d][:, oo * 128:(oo + 1) * 128],
                                        xt[:, d, c0:c0 + cw],
                                        start=(d == 0), stop=(d == ND - 1))
                                nc.vector.tensor_scalar_add(
                                    dstT[:, o, c0:c0 + cw], ps[:, :cw],
                                    bT_sb[:, proj * 8 + o:proj * 8 + o + 1])

        # ---- attention ----
        with ExitStack() as s4:
            pss = s4.enter_context(tc.tile_pool(name="pss", bufs=2, space="PSUM"))
            psc = s4.enter_context(tc.tile_pool(name="psc", bufs=1, space="PSUM"))
            tpp = s4.enter_context(tc.tile_pool(name="tpp", bufs=2, space="PSUM"))
            et_pool = s4.enter_context(tc.tile_pool(name="et", bufs=4))
            cs_pool = s4.enter_context(tc.tile_pool(name="cs", bufs=2))
            outp = s4.enter_context(tc.tile_pool(name="outp", bufs=5))
            rec_pool = s4.enter_context(tc.tile_pool(name="rec", bufs=4))

            for (c0, cw) in CHUNKS:
                sub = [(s0, min(128, cw - s0)) for s0 in range(0, cw, 128)]
                outs = []
                for _ in sub:
                    outs.append(outp.tile([128, D], F32, tag="out", name="out_sb"))
                for hp in range(8):
                    ps_c = psc.tile([65, 2, 512], F32, tag="psc", name="psc")
                    for kt in range(NT):
                        k0, ksz = kt * 128, TSZ[kt]
                        ps_s = pss.tile([128, 2, 512], F32, tag="pss", name="pss")
                        et = et_pool.tile([128, 2, 512], F32R, tag="et", name="et")
                        for hi in range(2):
                            p0 = hi * 64
                            nc.tensor.matmul(
                                ps_s[:ksz, hi, :cw],
                                kT[p0:p0 + 64, hp, k0:k0 + ksz],
                                qT[p0:p0 + 64, hp, c0:c0 + cw],
                                start=True, stop=True)
                        nc.scalar.activation(
                            et[:ksz, :, :cw], ps_s[:ksz, :, :cw], EXP, scale=0.125)
                        for hi in range(2):
                            h = 2 * hp + hi
                            nc.tensor.matmul(
                                ps_c[:, hi, :cw],
                                v_ext[:ksz, kt, h * 65:(h + 1) * 65],
                                et[:ksz, hi, :cw],
                                start=(kt == 0), stop=(kt == NT - 1))
                    for hi in range(2):
                        h = 2 * hp + hi
                        cst = cs_pool.tile([65, 512], F32, tag="cs", name="cs")
                        nc.vector.tensor_copy(cst[:, :cw], ps_c[:, hi, :cw])
                        for (si, (s0, ssz)) in enumerate(sub):
                            tp = tpp.tile([128, 65], F32, tag="tp", name="tp")
                            nc.tensor.transpose(
                                tp[:ssz, :], cst[:, s0:s0 + ssz], ident[:65, :65])
                            rec = rec_pool.tile([128, 1], F32, tag="rec", name="rec")
                            nc.vector.reciprocal(rec[:ssz], tp[:ssz, 64:65])
                            nc.vector.tensor_scalar_mul(
                                outs[si][:ssz, h * 64:(h + 1) * 64],
                                tp[:ssz, 0:64], rec[:ssz])
                for (si, (s0, ssz)) in enumerate(sub):
                    nc.sync.dma_start(
                        out[c0 + s0:c0 + s0 + ssz, :], outs[si][:ssz, :])


def build_program(reps=1):
    nc = bacc.Bacc("TRN2", target_bir_lowering=False, debug=False,
                   num_devices=8)
    xT = nc.dram_tensor("xT", [D, S], F32R, kind="ExternalInput").ap()
    wT = nc.dram_tensor("wT", [D, 3 * D], F32R, kind="ExternalInput").ap()
    bT = nc.dram_tensor("bT", [128, 24], F32, kind="ExternalInput").ap()
    bvb = nc.dram_tensor("bvb", [128, D], F32, kind="ExternalInput").ap()
    out = nc.dram_tensor("out", [S, D], F32, kind="ExternalOutput").ap()
    with tile.TileContext(nc) as tc:
        _body(tc, xT, wT, bT, bvb, out, reps=reps)
    nc.compile()
    return nc


_PROGRAM = None


def _get_program():
    global _PROGRAM
    if _PROGRAM is None:
        _PROGRAM = build_program()
    return _PROGRAM


def _prep_inputs(hidden_states, Wq, bq, Wk, bk, Wv, bv):
    hs = np.asarray(hidden_states, dtype=np.float32)
    B = hs.shape[0]
    xT = np.ascontiguousarray(hs.transpose(0, 2, 1))
    wT = np.ascontiguousarray(np.concatenate(
        [np.asarray(Wq, dtype=np.float32).T,
         np.asarray(Wk, dtype=np.float32).T,
         np.asarray(Wv, dtype=np.float32).T], axis=1))
    b_all = np.concatenate([np.asarray(bq, dtype=np.float32),
                            np.asarray(bk, dtype=np.float32),
                            np.asarray(bv, dtype=np.float32)])
    bT_np = np.ascontiguousarray(b_all.reshape(24, 128).T)
    bvb_np = np.ascontiguousarray(
        np.broadcast_to(np.asarray(bv, dtype=np.float32), (128, D)))
    return [{"xT": xT[b], "wT": wT, "bT": bT_np, "bvb": bvb_np}
            for b in range(B)]


def run(in_maps, **kw):
    nc = _get_program()
    return bass_utils.run_bass_kernel_spmd(
        nc, in_maps, core_ids=list(range(len(in_maps))), **kw)


def kernel(hidden_states, Wq, bq, Wk, bk, Wv, bv):
    in_maps = _prep_inputs(hidden_states, Wq, bq, Wk, bk, Wv, bv)
    res = run(in_maps)
    return np.stack([res.results[b]["out"] for b in range(len(in_maps))],
                    axis=0)


# revision 21
# speedup vs baseline: 2.2462x; 2.2462x over previous
"""DINOv2 self-attention (QKV projection + SDPA, no out-proj) on 8 Trainium2
NeuronCores.

Sharding: pure data-parallel over batch (B=8 -> one batch element per core);
no cross-core communication.

Host-side prep inside kernel(): transpose hidden_states to x.T per batch and
pack W as W.T = [Wq.T | Wk.T | Wv.T], so every on-chip matmul operand already
has its contraction dim on the partition axis.

Per-core program (S=1370, D=1024, H=16, hd=64), all matmuls in float32r
(TF32-like, full PE rate at moving-dim >= 256):
  1. qT = (x Wq^T + bq)^T, kT likewise: out.T = W.T^T @ x.T with W.T tiles
     stationary; bias added on DVE during PSUM->SBUF copy.  Layout [o, s]
     puts head_dim on partitions for attention.
  2. v in standard layout [s, o] (x.T tiles stationary, W.T moving), written
     interleaved with a ones-column per head: v_ext[:, t, h*65+64] = 1 so the
     softmax denominator falls out of the ctx matmul as an extra output row.
  3. Per head-pair (two heads share an o-tile at partition 0/64 -> concurrent
     PE row-groups), per sq-chunk (<=512): scoresT[sk, sq] = kT^T @ qT,
     exp via ACT with fused 1/8 scale (softmax max-subtraction skipped:
     |scores/8| <= ~6), ctx.T[65, sq] accumulated over the 11 sk tiles with
     [v | 1] stationary.
  4. ctx.T (+denominator row) transposed back with PE transpose per 128-wide
     sub-tile, then out[:, h*64:h*64+64] = ctx * (1/denom) on DVE.
"""

import numpy as np
from contextlib import ExitStack

import concourse.bass as bass
import concourse.bacc as bacc
import concourse.tile as tile
from concourse import mybir
from concourse import bass_utils
from concourse.masks import make_identity

S, D, H, HD = 1370, 1024, 16, 64
F32 = mybir.dt.float32
F32R = mybir.dt.float32r
ND = D // 128                      # 8 contraction tiles
NO = D // 128                      # 8 output tiles per projection
NT = (S + 127) // 128              # 11 token tiles
TSZ = [min(128, S - i * 128) for i in range(NT)]
CHUNKS = [(0, 512), (512, 512), (1024, S - 1024)]
EXP = mybir.ActivationFunctionType.Exp
CTX_LAG = False


def _body(tc, xT, wT, bT, bvb, out, reps=1):
    nc = tc.nc
    with ExitStack() as ctx:
        const = ctx.enter_context(tc.tile_pool(name="const", bufs=1))
        ident = const.tile([128, 128], F32)
        make_identity(nc, ident)
        bT_sb = const.tile([128, 24], F32)
        nc.sync.dma_start(bT_sb[:], bT[:])

        qk_pool = ctx.enter_context(tc.tile_pool(name="qkT", bufs=1))
        vext_pool = ctx.enter_context(tc.tile_pool(name="vext", bufs=1))
        for _rep in range(reps):
            _one_pass(tc, ctx, qk_pool, vext_pool, ident, bT_sb, bvb,
                      xT, wT, out)


def _one_pass(tc, ctx, qk_pool, vext_pool, ident, bT_sb, bvb, xT, wT, out):
        nc = tc.nc
        qT = qk_pool.tile([128, NO, S], F32R, tag="qT", name="qT")
        kT = qk_pool.tile([128, NO, S], F32R, tag="kT", name="kT")
        v_ext = vext_pool.tile([128, NT, H * 65], F32R, tag="vext", name="v_ext")
        # ones columns (h*65+64) for the fused softmax denominator; memset
        # can't produce f32r, so synthesize 1.0 as in0*0 + 1 on DVE
        for t in range(NT):
            ones_view = v_ext[:, t, :].rearrange("p (h e) -> p h e", e=65)[:, :, 64]
            nc.vector.tensor_scalar(
                ones_view, bT_sb[:, 0:16],
                0.0, 1.0, mybir.AluOpType.mult, mybir.AluOpType.add)

        with ExitStack() as s1:
            xt_pool = s1.enter_context(tc.tile_pool(name="xt", bufs=1))
            xt = xt_pool.tile([128, ND, S], F32R)
            for d in range(ND):
                nc.sync.dma_start(xt[:, d, :], xT[d * 128:(d + 1) * 128, :])

            # ---- v = x @ Wv^T + bv, scattered into v_ext ----
            with ExitStack() as s2:
                wv_pool = s2.enter_context(tc.tile_pool(name="wv", bufs=1))
                bvb_sb = wv_pool.tile([128, D], F32, tag="bvb", name="bvb_sb")
                nc.sync.dma_start(bvb_sb[:], bvb[:])
                psv = s2.enter_context(
                    tc.tile_pool(name="psv", bufs=4, space="PSUM"))
                for half in range(2):
                    wv = wv_pool.tile([128, ND, 512], F32R, tag="wv", name="wv")
                    for d in range(ND):
                        c = 2 * D + half * 512
                        nc.sync.dma_start(
                            wv[:, d, :], wT[d * 128:(d + 1) * 128, c:c + 512])
                    for t in range(NT):
                        tsz = TSZ[t]
                        ps = psv.tile([128, 512], F32, tag="psv", name="psv")
                        for d in range(ND):
                            nc.tensor.matmul(
                                ps[:tsz, :], xt[:, d, t * 128:t * 128 + tsz],
                                wv[:, d, :], start=(d == 0), stop=(d == ND - 1))
                        dst = v_ext[:tsz, t, :].rearrange(
                            "p (h e) -> p h e", e=65)[:, half * 8:(half + 1) * 8, 0:64]
                        src = ps[:tsz, :].rearrange("p (h e) -> p h e", e=64)
                        bias = bvb_sb[:tsz, half * 512:(half + 1) * 512].rearrange(
                            "p (h e) -> p h e", e=64)
                        nc.vector.tensor_add(dst, src, bias)

            # ---- qT / kT projections (W loaded 2 o-tiles per DMA) ----
            with ExitStack() as s3:
                wqk_pool = s3.enter_context(tc.tile_pool(name="wqk", bufs=9))
                psqk = s3.enter_context(
                    tc.tile_pool(name="psqk", bufs=4, space="PSUM"))
                for og in range(0, NO, 2):
                    for proj in (1, 0):
                        dstT = qT if proj == 0 else kT
                        ws = []
                        for d in range(ND):
                            w = wqk_pool.tile([128, 256], F32R,
                                              tag="wqk", name="wqk")
                            c = proj * D + og * 128
                            nc.sync.dma_start(
                                w[:], wT[d * 128:(d + 1) * 128, c:c + 256])
                            ws.append(w)
                        for oo in range(2):
                            o = og + oo
                            for (c0, cw) in CHUNKS:
                                ps = psqk.tile([128, 512], F32,
                                               tag="psqk", name="psqk")
                                for d in range(ND):
                                    nc.tensor.matmul(
                                        ps[:, :cw],
                                        ws[d][:, oo * 128:(oo + 1) * 128],
                                        xt[:, d, c0:c0 + cw],
                                        start=(d == 0), stop=(d == ND - 1))
                                nc.vector.tensor_scalar_add(
                                    dstT[:, o, c0:c0 + cw], ps[:, :cw],
                                    bT_sb[:, proj * 8 + o:proj * 8 + o + 1])

        # ---- attention ----
        with ExitStack() as s4:
            pss = s4.enter_context(tc.tile_pool(name="pss", bufs=2, space="PSUM"))
            psc = s4.enter_context(tc.tile_pool(name="psc", bufs=1, space="PSUM"))
            tpp = s4.enter_context(tc.tile_pool(name="tpp", bufs=2, space="PSUM"))
            et_pool = s4.enter_context(tc.tile_pool(name="et", bufs=4))
            cs_pool = s4.enter_context(tc.tile_pool(name="cs", bufs=2))
            outp = s4.enter_context(tc.tile_pool(name="outp", bufs=5))
            rec_pool = s4.enter_context(tc.tile_pool(name="rec", bufs=4))

            for (c0, cw) in CHUNKS:
                sub = [(s0, min(128, cw - s0)) for s0 in range(0, cw, 128)]
                outs = []
                for _ in sub:
                    outs.append(outp.tile([128, D], F32, tag="out", name="out_sb"))
                for hp in range(8):
                    ps_c = psc.tile([65, 2, 512], F32, tag="psc", name="psc")
                    ets = {}

                    def emit_ctx(kt):
                        ksz = TSZ[kt]
                        for hi in range(2):
                            h = 2 * hp + hi
                            nc.tensor.matmul(
                                ps_c[:, hi, :cw],
                                v_ext[:ksz, kt, h * 65:(h + 1) * 65],
                                ets.pop(kt)[:ksz, hi, :cw] if hi else
                                ets[kt][:ksz, hi, :cw],
                                start=(kt == 0), stop=(kt == NT - 1))

                    for kt in range(NT):
                        k0, ksz = kt * 128, TSZ[kt]
                        ps_s = pss.tile([128, 2, 512], F32, tag="pss", name="pss")
                        et = et_pool.tile([128, 2, 512], F32R, tag="et", name="et")
                        ets[kt] = et
                        for hi in range(2):
                            p0 = hi * 64
                            nc.tensor.matmul(
                                ps_s[:ksz, hi, :cw],
                                kT[p0:p0 + 64, hp, k0:k0 + ksz],
                                qT[p0:p0 + 64, hp, c0:c0 + cw],
                                start=True, stop=True)
                        nc.scalar.activation(
                            et[:ksz, :, :cw], ps_s[:ksz, :, :cw], EXP, scale=0.125)
                        # ctx lags one kt so the next scores pair reaches ACT
                        # before PE spends time on ctx
                        if CTX_LAG:
                            if kt > 0:
                                emit_ctx(kt - 1)
                        else:
                            emit_ctx(kt)
                    if CTX_LAG:
                        emit_ctx(NT - 1)
                    for hi in range(2):
                        h = 2 * hp + hi
                        cst = cs_pool.tile([65, 512], F32, tag="cs", name="cs")
                        nc.vector.tensor_copy(cst[:, :cw], ps_c[:, hi, :cw])
                        for (si, (s0, ssz)) in enumerate(sub):
                            tp = tpp.tile([128, 65], F32, tag="tp", name="tp")
                            nc.tensor.transpose(
                                tp[:ssz, :], cst[:, s0:s0 + ssz], ident[:65, :65])
                            rec = rec_pool.tile([128, 1], F32, tag="rec", name="rec")
                            nc.vector.reciprocal(rec[:ssz], tp[:ssz, 64:65])
                            nc.vector.tensor_scalar_mul(
                                outs[si][:ssz, h * 64:(h + 1) * 64],
                                tp[:ssz, 0:64], rec[:ssz])
                for (si, (s0, ssz)) in enumerate(sub):
                    nc.sync.dma_start(
                        out[c0 + s0:c0 + s0 + ssz, :], outs[si][:ssz, :])


def build_program(reps=1):
    nc = bacc.Bacc("TRN2", target_bir_lowering=False, debug=False,
                   num_devices=8)
    xT = nc.dram_tensor("xT", [D, S], F32R, kind="ExternalInput").ap()
    wT = nc.dram_tensor("wT", [D, 3 * D], F32R, kind="ExternalInput").ap()
    bT = nc.dram_tensor("bT", [128, 24], F32, kind="ExternalInput").ap()
    bvb = nc.dram_tensor("bvb", [128, D], F32, kind="ExternalInput").ap()
    out = nc.dram_tensor("out", [S, D], F32, kind="ExternalOutput").ap()
    with tile.TileContext(nc) as tc:
        _body(tc, xT, wT, bT, bvb, out, reps=reps)
    nc.compile()
    return nc


_PROGRAM = None


def _get_program():
    global _PROGRAM
    if _PROGRAM is None:
        _PROGRAM = build_program()
    return _PROGRAM


def _prep_inputs(hidden_states, Wq, bq, Wk, bk, Wv, bv):
    hs = np.asarray(hidden_states, dtype=np.float32)
    B = hs.shape[0]
    xT = np.ascontiguousarray(hs.transpose(0, 2, 1))
    wT = np.ascontiguousarray(np.concatenate(
        [np.asarray(Wq, dtype=np.float32).T,
         np.asarray(Wk, dtype=np.float32).T,
         np.asarray(Wv, dtype=np.float32).T], axis=1))
    b_all = np.concatenate([np.asarray(bq, dtype=np.float32),
                            np.asarray(bk, dtype=np.float32),
                            np.asarray(bv, dtype=np.float32)])
    bT_np = np.ascontiguousarray(b_all.reshape(24, 128).T)
    bvb_np = np.ascontiguousarray(
        np.broadcast_to(np.asarray(bv, dtype=np.float32), (128, D)))
    return [{"xT": xT[b], "wT": wT, "bT": bT_np, "bvb": bvb_np}
            for b in range(B)]


def run(in_maps, **kw):
    nc = _get_program()
    return bass_utils.run_bass_kernel_spmd(
        nc, in_maps, core_ids=list(range(len(in_maps))), **kw)


def kernel(hidden_states, Wq, bq, Wk, bk, Wv, bv):
    in_maps = _prep_inputs(hidden_states, Wq, bq, Wk, bk, Wv, bv)
    res = run(in_maps)
    return np.stack([res.results[b]["out"] for b in range(len(in_maps))],
                    axis=0)
